# revision 9
# baseline (speedup 1.0000x reference)
"""Batch-all triplet loss on 8 Trainium2 NeuronCores (Bass/Tile).

Math: with d = pairwise euclidean distance matrix of the B embeddings,
  loss = sum_{i,j,k valid} relu(d[i,j] - d[i,k] + margin) / (#positive + eps)
valid <=> i != j, labels[i] == labels[j], labels[i] != labels[k]
(the other distinctness constraints are implied by the label ones).

Sharding: anchors are grouped by class; each core hosts 2 classes in two
64-row blocks (data-driven gathers keep the single SPMD program uniform).

Per core, on device:
  prep:
  - one fused matmul group produces g[r, c] = dot(x_r, x_c) - 0.5*sq_c
    - 0.5*sq_r - BIG/2*[class(c)==class(r)] : the X contraction uses
    4x128 bf16 K-chunks; a 20-row tail chunk carries the anchor squared
    norm (hi+lo, stationary data vs moving ones), the column squared
    norm (hi+lo, stationary ones vs moving data) and 16 class-mask rows
    (stationary class-onehot vs moving -BIG/2 on own-class columns).
  - ACT Relu(scale=-2) then ACT Sqrt gives dmat (masked columns ~1e15);
    DVE tensor_scalar(-SHIFT) emits rhs_c[:, 0:B] (bf16, recentred).
  - the same structure over the partner columns gives bias[r, t] =
    d(anchor_r, t-th member of r's class) + margin - SHIFT, stored at
    rhs_c[:, B:B+T] (bf16).
  pair loop, NT tiles of 128 (anchor,positive) pairs:
  - PE replicates each pair's anchor row + bias row with a one-hot
    matmul into PSUM: rep[p, 0:B+T] = rhs_c[anchor(p), :].
  - Pool (gpsimd) builds the per-tile bias-select mask on the fly:
    sel[p, t] = (t == pos_idx[p, tau]), so no big select table is DMA'd.
  - DVE scalar_tensor_tensor(rep-bias-cols * sel) -> bias_p[p].
  - ACT: activation(Relu, in=rep[:, 0:B], scale=-1, bias=bias_p) ->
    o1 (bf16, SBUF) = relu(p - n) for 128 pairs x 640 columns.
  - per pair of tiles, two DVE tensor_scalar ops on the bf16 o1 ring run
    in 4x mode: sum (mult 1.0, accum) and count (is_gt 0, accum).
  - invalid negatives contribute 0 (the +1e15 mask), padded pairs are
    all-zero rows with bias 0 and contribute 0 to both sums.
  tail: free-dim reduce -> [128, 2], ones-matmul collapses partitions to
  a [1, 2] (sum, count) pair DMA'd out; host adds the 8 cores up.
"""

import numpy as np

import bass_rust
import concourse.bass as bass
import concourse.tile as tile
from concourse import mybir
from concourse.bass_utils import run_bass_kernel_spmd

N_CORES = 8
D_MODEL = 512
B_TOTAL = 640
N_CLASSES = 16
MARGIN = 0.3
EPS = 1e-8
RB = 64  # rows per block (max class size the device path supports)
BIG = 1.0e30
SHIFT = 32.0  # d values live in ~[27.6, 37]; recentring helps bf16
AUGR = 4 + N_CLASSES  # sqa_hi, sqa_lo, norm_hi, norm_lo, class masks
F32 = mybir.dt.float32
BF16 = mybir.dt.bfloat16
NP_BF16 = mybir.dt.np(mybir.dt.bfloat16)

_PROGRAM_CACHE: dict = {}


def _split_multi_waits(nc):
    """This toolchain's walrus codegen supports only ONE sync-wait per
    instruction; Tile can emit several. Move the extra waits onto
    same-engine NoOps inserted immediately before the instruction."""
    for func in nc.m.functions:
        for block in func.blocks:
            out = []
            for inst in block.instructions:
                si = inst.sync_info
                waits = list(si.on_wait) if si else []
                if len(waits) > 1:
                    for j, w in enumerate(waits[:-1]):
                        nop = mybir.InstNoOp(
                            name=f"{inst.name}-wsplit{j}", ins=[], outs=[]
                        )
                        nop.engine = inst.engine
                        nop.sync_info = bass_rust.SyncInfo(on_wait=[w], on_update=[])
                        out.append(nop)
                    inst.sync_info = bass_rust.SyncInfo(
                        on_wait=[waits[-1]], on_update=list(si.on_update)
                    )
                out.append(inst)
            block.instructions = out


def _build_program(B: int, D: int, T: int, NT: int):
    """One SPMD program for all 8 cores; per-core behavior is data-driven."""
    nc = bass.Bass()

    # combo columns: [0:B) moving operand (X^T), [B:B+128) stationary
    # anchor gather, [B+128:B+128+2T) positive-partner gather. Packed
    # DRAM layout [128, 4*W + NT + T]: chunk c of the K=512 contraction
    # occupies cols [c*W:(c+1)*W) (K-row c*128+p lives in partition p);
    # the per-pair positive index table (NT cols) and the 0..T-1 iota row
    # (T cols) ride along so the whole load is one fat-descriptor strided
    # stream. The 20-row tail chunk (norms + class masks) is `aug`.
    W = B + 128 + 2 * T
    CW = 4 * W + NT + T
    NS = (NT + 1) // 2  # super-tiles of 2 taus for the DVE accum ops
    combo = nc.declare_dram_parameter("combo", [128, CW], BF16, isOutput=False)
    aug = nc.declare_dram_parameter("aug", [AUGR, W], BF16, isOutput=False)
    onehot = nc.declare_dram_parameter("onehot", [128, NT * 128], BF16, isOutput=False)
    out_d = nc.declare_dram_parameter("out", [1, 2], F32, isOutput=True)

    WR = B + T  # replicated tile: negative columns + bias columns
    L0, L1 = B, B + 128  # stationary (anchor) columns within a chunk
    P0 = B + 128  # partner columns within a chunk

    with tile.TileContext(nc) as tc:
        with (
            tc.tile_pool(name="const", bufs=1) as const,
            tc.tile_pool(name="work", bufs=1) as work,
        ):
            # preload the ACT table set while DMAs run
            warm = const.tile([1, 8], F32)
            nc.vector.memset(warm, 1.0)
            nc.scalar.activation(out=warm, in_=warm,
                                 func=mybir.ActivationFunctionType.Sqrt)
            nc.scalar.activation(out=warm, in_=warm,
                                 func=mybir.ActivationFunctionType.Relu)

            # ---- input DMAs: aug first (tiny, needed by every matmul
            # group), then the combo stream as 16 fat per-partition-row
            # slices spread over all five issuing engines, then the
            # one-hot table in 4 column chunks (consumed progressively by
            # the pair loop). ----
            taug = const.tile([AUGR, W], BF16)
            nc.scalar.dma_start(out=taug, in_=aug[:, :])
            ktile = const.tile([128, CW], BF16)
            engines = [nc.sync, nc.scalar, nc.gpsimd]
            for i in range(16):
                p0, p1 = 8 * i, 8 * (i + 1)
                engines[i % 3].dma_start(
                    out=ktile[p0:p1, :], in_=combo[p0:p1, :]
                )
            t_oh = const.tile([128, NT * 128], BF16)
            ohq = (NT + 3) // 4 * 128
            for c in range(4):
                c0, c1 = c * ohq, min((c + 1) * ohq, NT * 128)
                if c0 < c1:
                    engines[c % 2].dma_start(out=t_oh[:, c0:c1], in_=onehot[:, c0:c1])

            kc = ktile[:, : 4 * W].rearrange("p (c w) -> p c w", c=4)
            t_iota = ktile[:, 4 * W + NT : 4 * W + NT + T]
            # AP scalar operands must be f32; widen the bf16 index table
            t_pos = const.tile([128, NT], F32)
            nc.vector.tensor_copy(t_pos, ktile[:, 4 * W : 4 * W + NT])

            rhs_c = work.tile([128, WR], BF16)  # [d_in | bias] combined

            with tc.tile_pool(name="psum_prep", bufs=1, space="PSUM") as psp:
                # ---- g = dot - 0.5*sq_col - 0.5*sq_anchor - class masks ----
                g = psp.tile([128, 1024], F32)
                for n0, n1 in [(0, 512), (512, B)]:
                    for ki in range(4):
                        nc.tensor.matmul(
                            g[:, n0:n1],
                            kc[:, ki, L0:L1],
                            kc[:, ki, n0:n1],
                            start=(ki == 0),
                            stop=False,
                        )
                    nc.tensor.matmul(
                        g[:, n0:n1], taug[:, L0:L1], taug[:, n0:n1],
                        start=False, stop=True,
                    )
                # ---- positive-pair distances: pb[r, t] ----
                pb = psp.tile([128, T], F32)
                for blk in range(2):
                    r0, r1 = blk * RB, (blk + 1) * RB
                    c0, c1 = P0 + blk * T, P0 + (blk + 1) * T
                    for ki in range(4):
                        nc.tensor.matmul(
                            pb[r0:r1, :],
                            kc[:, ki, L0 + r0 : L0 + r1],
                            kc[:, ki, c0:c1],
                            start=(ki == 0),
                            stop=False,
                        )
                    nc.tensor.matmul(
                        pb[r0:r1, :], taug[:, L0 + r0 : L0 + r1],
                        taug[:, c0:c1], start=False, stop=True,
                    )
                # bias = d_pos + margin - SHIFT -> rhs_c[:, B:B+T] (bf16)
                bsq = work.tile([128, T], F32)
                nc.scalar.activation(
                    out=bsq, in_=pb, func=mybir.ActivationFunctionType.Relu,
                    scale=-2.0,
                )
                bd = work.tile([128, T], F32)
                nc.scalar.activation(
                    out=bd, in_=bsq, func=mybir.ActivationFunctionType.Sqrt,
                )
                nc.vector.tensor_scalar(
                    out=rhs_c[:, B : B + T], in0=bd,
                    scalar1=float(MARGIN - SHIFT), scalar2=None,
                    op0=mybir.AluOpType.add,
                )
                # d_in = d - SHIFT (masked columns stay huge positive)
                dsq = work.tile([128, B], F32)
                nc.scalar.activation(
                    out=dsq, in_=g[:, 0:B],
                    func=mybir.ActivationFunctionType.Relu, scale=-2.0,
                )
                dmat = work.tile([128, B], F32)
                nc.scalar.activation(
                    out=dmat, in_=dsq, func=mybir.ActivationFunctionType.Sqrt,
                )
                nc.vector.tensor_scalar(
                    out=rhs_c[:, 0:B], in0=dmat,
                    scalar1=-SHIFT, scalar2=None,
                    op0=mybir.AluOpType.add,
                )

            # ---- pair loop ----
            sum_cols = work.tile([128, NS], F32)
            cnt_cols = work.tile([128, NS], F32)
            scr = work.tile([128, 2 * B], BF16)  # throwaway out of accum ops
            with (
                tc.tile_pool(name="psum_loop", bufs=3, space="PSUM") as psl,
                tc.tile_pool(name="psum_tail", bufs=1, space="PSUM") as pst,
                tc.tile_pool(name="selp", bufs=3) as selp,
                tc.tile_pool(name="bp", bufs=3) as bpp,
                tc.tile_pool(name="ttrp", bufs=2) as ttrp,
                tc.tile_pool(name="o1p", bufs=2) as o1p,
            ):
                for s in range(NS):
                    taus = [t for t in (2 * s, 2 * s + 1) if t < NT]
                    o1 = o1p.tile([128, 2, B], BF16, tag="o1")
                    for half, tau in enumerate(taus):
                        # Pool: sel[p, t] = (t == pos_idx[p, tau])
                        sel = selp.tile([128, T], BF16, tag="sel")
                        nc.gpsimd.tensor_scalar(
                            out=sel, in0=t_iota,
                            scalar1=t_pos[:, tau : tau + 1], scalar2=None,
                            op0=mybir.AluOpType.is_equal,
                        )
                        # PE: replicate anchor row + bias row per pair
                        rep = psl.tile([128, 1024], F32, tag="rep")
                        oh = t_oh[:, tau * 128 : (tau + 1) * 128]
                        nc.tensor.matmul(
                            rep[:, 0:512], oh, rhs_c[:, 0:512],
                            start=True, stop=True,
                        )
                        nc.tensor.matmul(
                            rep[:, 512:WR], oh, rhs_c[:, 512:WR],
                            start=True, stop=True,
                        )
                        # DVE: extract this pair's bias
                        bias_p = bpp.tile([128, 1], F32, tag="bias_p")
                        ttr = ttrp.tile([128, T], F32, tag="ttr")
                        nc.vector.scalar_tensor_tensor(
                            out=ttr, in0=rep[:, B:WR], scalar=1.0,
                            in1=sel,
                            op0=mybir.AluOpType.mult, op1=mybir.AluOpType.mult,
                            accum_out=bias_p,
                        )
                        # ACT: o1 = relu(bias_p - rep) (bf16, SBUF)
                        nc.scalar.activation(
                            out=o1[:, half, :], in_=rep[:, 0:B],
                            func=mybir.ActivationFunctionType.Relu,
                            bias=bias_p, scale=-1.0,
                        )
                    # DVE on the bf16 ring (4x mode): sum and count
                    span = o1[:, 0 : len(taus), :]
                    nc.vector.tensor_scalar(
                        out=scr[:, 0 : len(taus) * B], in0=span,
                        scalar1=1.0, scalar2=0.0,
                        op0=mybir.AluOpType.mult, op1=mybir.AluOpType.add,
                        accum_out=sum_cols[:, s : s + 1],
                    )
                    nc.vector.tensor_scalar(
                        out=scr[:, 0 : len(taus) * B], in0=span,
                        scalar1=0.0, scalar2=0.0,
                        op0=mybir.AluOpType.is_gt, op1=mybir.AluOpType.add,
                        accum_out=cnt_cols[:, s : s + 1],
                    )

                # ---- tail: reduce, collapse partitions, write [1,2] ----
                stat = work.tile([128, 2], F32)
                nc.vector.tensor_reduce(
                    out=stat[:, 0:1], in_=sum_cols, axis=mybir.AxisListType.X,
                    op=mybir.AluOpType.add,
                )
                nc.vector.tensor_reduce(
                    out=stat[:, 1:2], in_=cnt_cols, axis=mybir.AxisListType.X,
                    op=mybir.AluOpType.add,
                )
                ones = work.tile([128, 1], F32)
                nc.vector.memset(ones, 1.0)
                tot = pst.tile([1, 2], F32, tag="tot")
                nc.tensor.matmul(tot, ones, stat, start=True, stop=True)
                tot_s = work.tile([1, 2], F32)
                nc.vector.tensor_copy(tot_s, tot)
                nc.sync.dma_start(out=out_d[:, :], in_=tot_s)

    _split_multi_waits(nc)
    return nc


def _schedule(labels: np.ndarray):
    """Group anchors by class, pair classes onto cores (big with small)."""
    vals, counts = np.unique(labels, return_counts=True)
    classes = [np.nonzero(labels == v)[0] for v in vals]
    order = np.argsort(-counts, kind="stable")
    classes = [classes[i] for i in order]
    sizes = [len(c) for c in classes]
    if len(classes) > 2 * N_CORES or max(sizes) > RB:
        return None  # device path infeasible for this label layout
    while len(classes) < 2 * N_CORES:
        classes.append(np.zeros((0,), dtype=np.int64))
    blocks = []
    for i in range(N_CORES):
        blocks.append((classes[i], classes[2 * N_CORES - 1 - i]))
    T = max(1, max(len(c) for c, _ in blocks))
    npairs = [len(a) * (len(a) - 1) + len(b) * (len(b) - 1) for a, b in blocks]
    NT = max(1, (max(npairs) + 127) // 128)
    return blocks, T, NT


def _host_fallback(X: np.ndarray, labels: np.ndarray) -> np.float32:
    """Exact numpy implementation (only for label layouts the device
    schedule cannot represent — cannot occur for randint(0,16) labels)."""
    Xd = X.astype(np.float64)
    dot = Xd @ Xd.T
    sq = np.diag(dot).copy()
    dm = np.maximum(sq[None, :] - 2.0 * dot + sq[:, None], 0.0)
    zero = dm == 0.0
    dm = np.sqrt(dm + zero * EPS) * (1.0 - zero)
    total = 0.0
    npos = 0
    B = len(labels)
    for i in range(B):
        pos = (labels == labels[i]) & (np.arange(B) != i)
        neg = labels != labels[i]
        p = dm[i, pos] + MARGIN
        n = dm[i, neg]
        tl = np.maximum(p[:, None] - n[None, :], 0.0)
        total += tl.sum()
        npos += (tl > EPS).sum()
    return np.float32(total / (npos + EPS))


def _hi_lo(x: np.ndarray):
    hi = x.astype(NP_BF16)
    lo = (x - hi.astype(np.float32)).astype(NP_BF16)
    return hi, lo


def _make_in_maps(X: np.ndarray, lab: np.ndarray, blocks, T: int, NT: int):
    B, D = X.shape
    sq = (X.astype(np.float64) ** 2).sum(axis=1).astype(np.float32)
    W = B + 128 + 2 * T
    CW = 4 * W + NT + T

    XT = np.ascontiguousarray(X.T)
    sq_hi, sq_lo = _hi_lo(-0.5 * sq)
    in_maps = []
    for core in range(N_CORES):
        cls_a, cls_b = blocks[core]
        row_idx = np.zeros(128, dtype=np.int64)
        for blk, cls in enumerate((cls_a, cls_b)):
            m = len(cls)
            r0 = blk * RB
            if m:
                row_idx[r0 : r0 + m] = cls
                row_idx[r0 + m : r0 + RB] = cls[0]

        par_idx = np.zeros(2 * T, dtype=np.int64)
        for blk, cls in enumerate((cls_a, cls_b)):
            m = len(cls)
            c0 = blk * T
            if m:
                par_idx[c0 : c0 + m] = cls

        # X part of the contraction, packed [128, 4, W]
        xcols = np.concatenate(
            [XT, XT[:, row_idx], XT[:, par_idx]], axis=1
        ).astype(NP_BF16)  # [D, W]
        packed = np.ascontiguousarray(
            xcols.reshape(4, 128, W).transpose(1, 0, 2)
        ).reshape(128, 4 * W)

        # aug rows: anchor norms (stationary data x moving ones), column
        # norms (stationary ones x moving data), 16 class-mask rows
        # (stationary class-onehot x moving -BIG/2 on own-class columns)
        aug = np.zeros((AUGR, W), dtype=np.float32)
        aug[0, :], aug[1, :] = 1.0, 1.0
        aug[0, B : B + 128] = -0.5 * sq[row_idx]
        aug[1, B : B + 128] = (
            -0.5 * sq[row_idx]
            - aug[0, B : B + 128].astype(NP_BF16).astype(np.float32)
        )
        aug[2, 0:B] = sq_hi
        aug[3, 0:B] = sq_lo
        aug[2, B + 128 :] = sq_hi[par_idx]
        aug[3, B + 128 :] = sq_lo[par_idx]
        aug[2, B : B + 128] = 1.0
        aug[3, B : B + 128] = 1.0
        for c in range(N_CLASSES):
            aug[4 + c, 0:B] = np.where(lab == c, -0.5 * BIG, 0.0)
            aug[4 + c, B : B + 128] = (lab[row_idx] == c).astype(np.float32)
        aug_bf = aug.astype(NP_BF16)
        # keep the hi+lo split exact after the bf16 round-trip
        aug_bf[1, B : B + 128] = (
            -0.5 * sq[row_idx]
            - aug_bf[0, B : B + 128].astype(np.float32)
        ).astype(NP_BF16)

        # pair tables: one-hot anchor pick + positive index per pair
        onehot = np.zeros((128, NT * 128), dtype=NP_BF16)
        pos_idx = np.full((128, NT), -1.0, dtype=NP_BF16)
        p = 0
        for blk, cls in enumerate((cls_a, cls_b)):
            m = len(cls)
            r0 = blk * RB
            for i in range(m):
                for t in range(m):
                    if t == i:
                        continue
                    tau, q = divmod(p, 128)
                    onehot[r0 + i, tau * 128 + q] = 1.0
                    pos_idx[q, tau] = float(t)
                    p += 1
        assert p <= NT * 128

        combo = np.zeros((128, CW), dtype=NP_BF16)
        combo[:, : 4 * W] = packed
        combo[:, 4 * W : 4 * W + NT] = pos_idx
        combo[:, 4 * W + NT :] = np.arange(T, dtype=np.float32)[None, :]

        in_maps.append({"combo": combo, "aug": aug_bf, "onehot": onehot})
    return in_maps


def kernel(embeddings: np.ndarray, labels: np.ndarray) -> np.ndarray:
    X = np.ascontiguousarray(np.asarray(embeddings), dtype=np.float32)
    lab = np.asarray(labels).astype(np.int64)
    B, D = X.shape
    assert B == B_TOTAL and D == D_MODEL, (B, D)

    sched = _schedule(lab)
    if sched is None:
        return _host_fallback(X, lab)
    blocks, T, NT = sched
    in_maps = _make_in_maps(X, lab, blocks, T, NT)

    key = (B, D, T, NT)
    nc = _PROGRAM_CACHE.get(key)
    if nc is None:
        nc = _build_program(B, D, T, NT)
        _PROGRAM_CACHE[key] = nc

    res = run_bass_kernel_spmd(nc, in_maps, core_ids=list(range(N_CORES)))
    total_sum = 0.0
    total_cnt = 0.0
    for r in res.results:
        o = np.asarray(r["out"], dtype=np.float64)
        total_sum += o[0, 0]
        total_cnt += o[0, 1]
    return np.float32(total_sum / (total_cnt + EPS))


# revision 12
# speedup vs baseline: 1.0071x; 1.0071x over previous
"""Batch-all triplet loss on 8 Trainium2 NeuronCores (Bass/Tile).

Math: with d = pairwise euclidean distance matrix of the B embeddings,
  loss = sum_{i,j,k valid} relu(d[i,j] - d[i,k] + margin) / (#positive + eps)
valid <=> i != j, labels[i] == labels[j], labels[i] != labels[k]
(the other distinctness constraints are implied by the label ones).

Sharding: anchors are grouped by class; each core hosts 2 classes in two
64-row blocks (data-driven gathers keep the single SPMD program uniform).

Per core, on device:
  prep:
  - one fused matmul group produces g[r, c] = dot(x_r, x_c) - 0.5*sq_c
    - 0.5*sq_r - MASK*[class(c)==class(r)] : the X contraction runs in
    fp8 (4x128 K-chunks; the stochastic fp8 dot error ~0.02 on d is far
    inside the 2e-2 gate and halves the gating DMA bytes); a 20-row bf16
    tail chunk carries the anchor squared norm (hi+lo, stationary data
    vs moving ones), the column squared norm (hi+lo, moving data vs
    stationary ones) and 16 class-mask rows (+-240 products).
  - ACT Relu(scale=-2) then ACT Sqrt gives dmat (masked columns ~338);
    DVE tensor_scalar(-SHIFT) emits rhs_c[:, 0:B] (bf16, recentred).
  - the same structure over the partner columns gives bias[r, t] =
    d(anchor_r, t-th member of r's class) + margin - SHIFT at
    rhs_c[:, B:B+T] (bf16).
  pair loop, NT tiles of 128 (anchor,positive) pairs (software-pipelined
  so each engine's in-order stream never stalls):
  - PE replicates each pair's anchor row + bias row with a one-hot
    matmul into PSUM: rep[p, 0:B+T] = rhs_c[anchor(p), :].
  - DVE builds the bias-select mask on the fly (sel[p,t] =
    (t == pos_idx[p, tau]), 4x mode) -> scalar_tensor_tensor extracts
    bias_p; ACT Relu(scale=-1, bias=bias_p) -> o1 (bf16 SBUF);
    DVE is_gt -> o2 (bf16, 4x mode).
  - PE reduces o1 and o2 with ones-matmuls into two persistent PSUM
    accumulator regions (column-wrapped, one accumulation group each
    across all NT tiles) - the DVE reduce path is 1x-capped and slower.
  - invalid negatives contribute 0 (the +306 mask), padded pairs are
    all-zero rows with bias 0 and contribute 0 to both sums.
  tail: one strided tensor_reduce of the [1, 2x512] accumulator ->
  [1, 2] (sum, count) DMA'd out; host adds the 8 cores up.
"""

import numpy as np

import bass_rust
import concourse.bass as bass
import concourse.tile as tile
from concourse import mybir
from concourse.bass_utils import run_bass_kernel_spmd

N_CORES = 8
D_MODEL = 512
B_TOTAL = 640
N_CLASSES = 16
MARGIN = 0.3
EPS = 1e-8
RB = 64  # rows per block (max class size the device path supports)
MASKQ = 240.0  # class-mask factor; product 57600 pushes masked d to ~338
SHIFT = 32.0  # d values live in ~[27.6, 37]; recentring helps bf16
AUGR = 4 + N_CLASSES  # sqa_hi, sqa_lo, norm_hi, norm_lo, class masks
TP = 48  # padded iota/sel width (even for DVE 4x mode)
F32 = mybir.dt.float32
BF16 = mybir.dt.bfloat16
FP8 = mybir.dt.float8e4
NP_BF16 = mybir.dt.np(mybir.dt.bfloat16)
NP_FP8 = mybir.dt.np(mybir.dt.float8e4)

_PROGRAM_CACHE: dict = {}


def _split_multi_waits(nc):
    """This toolchain's walrus codegen supports only ONE sync-wait per
    instruction; Tile can emit several. Move the extra waits onto
    same-engine NoOps inserted immediately before the instruction."""
    for func in nc.m.functions:
        for block in func.blocks:
            out = []
            for inst in block.instructions:
                si = inst.sync_info
                waits = list(si.on_wait) if si else []
                if len(waits) > 1:
                    for j, w in enumerate(waits[:-1]):
                        nop = mybir.InstNoOp(
                            name=f"{inst.name}-wsplit{j}", ins=[], outs=[]
                        )
                        nop.engine = inst.engine
                        nop.sync_info = bass_rust.SyncInfo(on_wait=[w], on_update=[])
                        out.append(nop)
                    inst.sync_info = bass_rust.SyncInfo(
                        on_wait=[waits[-1]], on_update=list(si.on_update)
                    )
                out.append(inst)
            block.instructions = out


def _build_program(B: int, D: int, T: int, NT: int):
    """One SPMD program for all 8 cores; per-core behavior is data-driven."""
    nc = bass.Bass()

    # xcombo columns: [0:B) moving operand (X^T), [B:B+128) stationary
    # anchor gather, [B+128:B+128+2T) positive-partner gather, all fp8.
    # Packed DRAM layout [128, 4*W]: chunk c of the K=512 contraction
    # occupies cols [c*W:(c+1)*W) (K-row c*128+p lives in partition p).
    W = B + 128 + 2 * T
    xcombo = nc.declare_dram_parameter("xcombo", [128, 4 * W], FP8, isOutput=False)
    aug = nc.declare_dram_parameter("aug", [AUGR, W], BF16, isOutput=False)
    ptab = nc.declare_dram_parameter("ptab", [128, NT + TP], BF16, isOutput=False)
    # one-hot stationary, shipped with only the 2*T meaningful rows
    onehot = nc.declare_dram_parameter(
        "onehot", [2 * T, NT * 128], BF16, isOutput=False
    )
    out_d = nc.declare_dram_parameter("out", [1, 2], F32, isOutput=True)

    WR = B + T  # replicated tile: negative columns + bias columns
    L0, L1 = B, B + 128  # stationary (anchor) columns within a chunk
    P0 = B + 128  # partner columns within a chunk

    with tile.TileContext(nc) as tc:
        with (
            tc.tile_pool(name="const", bufs=1) as const,
            tc.tile_pool(name="work", bufs=1) as work,
        ):
            # preload the ACT table set while DMAs run
            warm = const.tile([1, 8], F32)
            nc.vector.memset(warm, 1.0)
            nc.scalar.activation(out=warm, in_=warm,
                                 func=mybir.ActivationFunctionType.Sqrt)
            nc.scalar.activation(out=warm, in_=warm,
                                 func=mybir.ActivationFunctionType.Relu)

            # ---- input DMAs. sync's queue carries the two big streams
            # (xcombo first - it gates prep - then the one-hot rows);
            # scalar's queue carries the small tables in parallel. ----
            ktile = const.tile([128, 4 * W], FP8)
            nc.sync.dma_start(out=ktile, in_=xcombo[:, :])
            taug = const.tile([AUGR, W], BF16)
            nc.scalar.dma_start(out=taug, in_=aug[:, :])
            tptab = const.tile([128, NT + TP], BF16)
            nc.scalar.dma_start(out=tptab, in_=ptab[:, :])
            t_oh = const.tile([128, NT * 128], BF16)
            if T < RB:  # zero the pad rows the gathers skip
                nc.vector.memset(t_oh, 0.0)
            nc.sync.dma_start(out=t_oh[0:T, :], in_=onehot[0:T, :])
            nc.sync.dma_start(out=t_oh[RB : RB + T, :], in_=onehot[T : 2 * T, :])

            kc = ktile.rearrange("p (c w) -> p c w", c=4)
            t_iota = tptab[:, NT : NT + TP]
            # AP scalar operands must be f32; widen the bf16 index table
            t_pos = const.tile([128, NT], F32)
            nc.vector.tensor_copy(t_pos, tptab[:, 0:NT])

            rhs_c = work.tile([128, WR], BF16)  # [d_in | bias] combined
            ones = work.tile([128, 1], BF16)
            nc.vector.memset(ones, 1.0)

            with tc.tile_pool(name="psum_prep", bufs=1, space="PSUM") as psp:
                # ---- g = dot - 0.5*sq_col - 0.5*sq_anchor - class masks ----
                g = psp.tile([128, 1024], F32)
                for n0, n1 in [(0, 512), (512, B)]:
                    for ki in range(4):
                        nc.tensor.matmul(
                            g[:, n0:n1],
                            kc[:, ki, L0:L1],
                            kc[:, ki, n0:n1],
                            start=(ki == 0),
                            stop=False,
                        )
                    nc.tensor.matmul(
                        g[:, n0:n1], taug[:, L0:L1], taug[:, n0:n1],
                        start=False, stop=True,
                    )
                # ---- positive-pair distances: pb[r, t] ----
                pb = psp.tile([128, T], F32)
                for blk in range(2):
                    r0, r1 = blk * RB, (blk + 1) * RB
                    c0, c1 = P0 + blk * T, P0 + (blk + 1) * T
                    for ki in range(4):
                        nc.tensor.matmul(
                            pb[r0:r1, :],
                            kc[:, ki, L0 + r0 : L0 + r1],
                            kc[:, ki, c0:c1],
                            start=(ki == 0),
                            stop=False,
                        )
                    nc.tensor.matmul(
                        pb[r0:r1, :], taug[:, L0 + r0 : L0 + r1],
                        taug[:, c0:c1], start=False, stop=True,
                    )
                # bias = d_pos + margin - SHIFT -> rhs_c[:, B:B+T] (bf16)
                bsq = work.tile([128, T], F32)
                nc.scalar.activation(
                    out=bsq, in_=pb, func=mybir.ActivationFunctionType.Relu,
                    scale=-2.0,
                )
                bd = work.tile([128, T], F32)
                nc.scalar.activation(
                    out=bd, in_=bsq, func=mybir.ActivationFunctionType.Sqrt,
                )
                nc.vector.tensor_scalar(
                    out=rhs_c[:, B : B + T], in0=bd,
                    scalar1=float(MARGIN - SHIFT), scalar2=None,
                    op0=mybir.AluOpType.add,
                )
                # d_in = d - SHIFT (masked columns stay ~306)
                dsq = work.tile([128, B], F32)
                nc.scalar.activation(
                    out=dsq, in_=g[:, 0:B],
                    func=mybir.ActivationFunctionType.Relu, scale=-2.0,
                )
                dmat = work.tile([128, B], F32)
                nc.scalar.activation(
                    out=dmat, in_=dsq, func=mybir.ActivationFunctionType.Sqrt,
                )
                nc.vector.tensor_scalar(
                    out=rhs_c[:, 0:B], in0=dmat,
                    scalar1=-SHIFT, scalar2=None,
                    op0=mybir.AluOpType.add,
                )

            # ---- pair loop (software-pipelined: PE reduce of tile t is
            # emitted during tile t+1 / t+2 so in-order engine streams
            # never wait on cross-engine producers) ----
            with (
                tc.tile_pool(name="psum_loop", bufs=3, space="PSUM") as psl,
                tc.tile_pool(name="psum_acc", bufs=1, space="PSUM") as psa,
                tc.tile_pool(name="selp", bufs=3) as selp,
                tc.tile_pool(name="bp", bufs=3) as bpp,
                tc.tile_pool(name="ttrp", bufs=2) as ttrp,
                tc.tile_pool(name="o1p", bufs=3) as o1p,
                tc.tile_pool(name="o2p", bufs=3) as o2p,
            ):
                # acc[0, 0:512] accumulates the o1 sums (column-wrapped),
                # acc[0, 512:1024] the o2 counts, across all NT tiles.
                acc = psa.tile([1, 1024], F32)
                reps, o1s, o2s = {}, {}, {}

                def emit_sum(t, last):
                    o1 = o1s.pop(t)
                    nc.tensor.matmul(
                        acc[0:1, 0:512], ones, o1[:, 0:512],
                        start=(t == 0), stop=False, skip_group_check=True,
                    )
                    nc.tensor.matmul(
                        acc[0:1, 0:128], ones, o1[:, 512:B],
                        start=False, stop=last, skip_group_check=True,
                    )

                def emit_cnt(t, last):
                    o2 = o2s.pop(t)
                    nc.tensor.matmul(
                        acc[0:1, 512:1024], ones, o2[:, 0:512],
                        start=(t == 0), stop=False, skip_group_check=True,
                    )
                    nc.tensor.matmul(
                        acc[0:1, 512:640], ones, o2[:, 512:B],
                        start=False, stop=last, skip_group_check=True,
                    )

                def emit_isgt(t):
                    o2 = o2p.tile([128, B], BF16, tag="o2")
                    nc.vector.tensor_scalar(
                        out=o2, in0=o1s[t], scalar1=0.0, scalar2=None,
                        op0=mybir.AluOpType.is_gt,
                    )
                    o2s[t] = o2

                for tau in range(NT):
                    # DVE: sel[p, t] = (t == pos_idx[p, tau])
                    sel = selp.tile([128, TP], BF16, tag="sel")
                    nc.vector.tensor_scalar(
                        out=sel, in0=t_iota,
                        scalar1=t_pos[:, tau : tau + 1], scalar2=None,
                        op0=mybir.AluOpType.is_equal,
                    )
                    # PE: replicate anchor row + bias row per pair
                    rep = psl.tile([128, 1024], F32, tag="rep")
                    oh = t_oh[:, tau * 128 : (tau + 1) * 128]
                    nc.tensor.matmul(
                        rep[:, 0:512], oh, rhs_c[:, 0:512],
                        start=True, stop=True,
                    )
                    nc.tensor.matmul(
                        rep[:, 512:WR], oh, rhs_c[:, 512:WR],
                        start=True, stop=True,
                    )
                    # DVE: extract this pair's bias
                    bias_p = bpp.tile([128, 1], F32, tag="bias_p")
                    ttr = ttrp.tile([128, T], F32, tag="ttr")
                    nc.vector.scalar_tensor_tensor(
                        out=ttr, in0=rep[:, B:WR], scalar=1.0,
                        in1=sel[:, 0:T],
                        op0=mybir.AluOpType.mult, op1=mybir.AluOpType.mult,
                        accum_out=bias_p,
                    )
                    # ACT: o1 = relu(bias_p - rep) (bf16, SBUF)
                    o1 = o1p.tile([128, B], BF16, tag="o1")
                    nc.scalar.activation(
                        out=o1, in_=rep[:, 0:B],
                        func=mybir.ActivationFunctionType.Relu,
                        bias=bias_p, scale=-1.0,
                    )
                    o1s[tau] = o1
                    # deferred stages (pipeline depth 1 and 2)
                    if tau >= 1:
                        emit_isgt(tau - 1)
                        emit_sum(tau - 1, last=False)
                    if tau >= 2:
                        emit_cnt(tau - 2, last=False)
                emit_isgt(NT - 1)
                emit_sum(NT - 1, last=True)
                if NT >= 2:
                    emit_cnt(NT - 2, last=False)
                emit_cnt(NT - 1, last=True)

                # ---- tail: strided reduce -> [1, 2] -> DMA out ----
                tot = work.tile([1, 2], F32)
                nc.vector.tensor_reduce(
                    out=tot,
                    in_=acc[0:1, :].rearrange("p (c k) -> p c k", c=2),
                    axis=mybir.AxisListType.X,
                    op=mybir.AluOpType.add,
                )
                nc.sync.dma_start(out=out_d[:, :], in_=tot)

    _split_multi_waits(nc)
    return nc


def _schedule(labels: np.ndarray):
    """Group anchors by class, pair classes onto cores (big with small)."""
    vals, counts = np.unique(labels, return_counts=True)
    classes = [np.nonzero(labels == v)[0] for v in vals]
    order = np.argsort(-counts, kind="stable")
    classes = [classes[i] for i in order]
    sizes = [len(c) for c in classes]
    if len(classes) > 2 * N_CORES or max(sizes) > RB:
        return None  # device path infeasible for this label layout
    while len(classes) < 2 * N_CORES:
        classes.append(np.zeros((0,), dtype=np.int64))
    blocks = []
    for i in range(N_CORES):
        blocks.append((classes[i], classes[2 * N_CORES - 1 - i]))
    T = max(1, max(len(c) for c, _ in blocks))
    npairs = [len(a) * (len(a) - 1) + len(b) * (len(b) - 1) for a, b in blocks]
    NT = max(1, (max(npairs) + 127) // 128)
    return blocks, T, NT


def _host_fallback(X: np.ndarray, labels: np.ndarray) -> np.float32:
    """Exact numpy implementation (only for label layouts the device
    schedule cannot represent — cannot occur for randint(0,16) labels)."""
    Xd = X.astype(np.float64)
    dot = Xd @ Xd.T
    sq = np.diag(dot).copy()
    dm = np.maximum(sq[None, :] - 2.0 * dot + sq[:, None], 0.0)
    zero = dm == 0.0
    dm = np.sqrt(dm + zero * EPS) * (1.0 - zero)
    total = 0.0
    npos = 0
    B = len(labels)
    for i in range(B):
        pos = (labels == labels[i]) & (np.arange(B) != i)
        neg = labels != labels[i]
        p = dm[i, pos] + MARGIN
        n = dm[i, neg]
        tl = np.maximum(p[:, None] - n[None, :], 0.0)
        total += tl.sum()
        npos += (tl > EPS).sum()
    return np.float32(total / (npos + EPS))


def _make_in_maps(X: np.ndarray, lab: np.ndarray, blocks, T: int, NT: int):
    B, D = X.shape
    sq = (X.astype(np.float64) ** 2).sum(axis=1).astype(np.float32)
    W = B + 128 + 2 * T

    X8 = X.astype(NP_FP8)
    XT8 = np.ascontiguousarray(X8.T)
    sq_hi = (-0.5 * sq).astype(NP_BF16)
    sq_lo = ((-0.5 * sq) - sq_hi.astype(np.float32)).astype(NP_BF16)
    in_maps = []
    for core in range(N_CORES):
        cls_a, cls_b = blocks[core]
        row_idx = np.zeros(128, dtype=np.int64)
        for blk, cls in enumerate((cls_a, cls_b)):
            m = len(cls)
            r0 = blk * RB
            if m:
                row_idx[r0 : r0 + m] = cls
                row_idx[r0 + m : r0 + RB] = cls[0]

        par_idx = np.zeros(2 * T, dtype=np.int64)
        for blk, cls in enumerate((cls_a, cls_b)):
            m = len(cls)
            if m:
                par_idx[blk * T : blk * T + m] = cls

        # X part of the contraction (fp8), packed [128, 4, W]
        xcols = np.concatenate([XT8, XT8[:, row_idx], XT8[:, par_idx]], axis=1)
        packed = np.ascontiguousarray(
            xcols.reshape(4, 128, W).transpose(1, 0, 2)
        ).reshape(128, 4 * W)

        # aug rows (bf16): anchor norms (stationary data x moving ones),
        # column norms (stationary ones x moving data), 16 class-mask
        # rows (stationary 240*onehot x moving -240 on own-class columns)
        L0, L1 = B, B + 128
        augt = np.zeros((AUGR, W), dtype=np.float32)
        augt[0:4, :] = 1.0
        augt[0, L0:L1] = sq_hi[row_idx].astype(np.float32)
        augt[1, L0:L1] = (
            -0.5 * sq[row_idx] - sq_hi[row_idx].astype(np.float32)
        )
        augt[2, 0:B] = sq_hi.astype(np.float32)
        augt[3, 0:B] = sq_lo.astype(np.float32)
        augt[2, L1:W] = sq_hi[par_idx].astype(np.float32)
        augt[3, L1:W] = sq_lo[par_idx].astype(np.float32)
        for c in range(N_CLASSES):
            augt[4 + c, 0:B] = np.where(lab == c, -MASKQ, 0.0)
            augt[4 + c, L0:L1] = (lab[row_idx] == c) * MASKQ
        aug_bf = augt.astype(NP_BF16)
        # keep the anchor-norm hi+lo split exact after the bf16 round
        aug_bf[1, L0:L1] = (
            -0.5 * sq[row_idx] - aug_bf[0, L0:L1].astype(np.float32)
        ).astype(NP_BF16)

        # pair tables: one-hot anchor pick (compact: 2*T used rows) and
        # positive index per pair, plus the 0..TP-1 iota row
        onehot = np.zeros((2 * T, NT * 128), dtype=NP_BF16)
        ptab = np.zeros((128, NT + TP), dtype=NP_BF16)
        ptab[:, 0:NT] = -1.0
        ptab[:, NT:] = np.arange(TP, dtype=np.float32)[None, :]
        p = 0
        for blk, cls in enumerate((cls_a, cls_b)):
            m = len(cls)
            for i in range(m):
                for t in range(m):
                    if t == i:
                        continue
                    tau, q = divmod(p, 128)
                    onehot[blk * T + i, tau * 128 + q] = 1.0
                    ptab[q, tau] = float(t)
                    p += 1
        assert p <= NT * 128

        in_maps.append(
            {"xcombo": packed, "aug": aug_bf, "ptab": ptab, "onehot": onehot}
        )
    return in_maps


def kernel(embeddings: np.ndarray, labels: np.ndarray) -> np.ndarray:
    X = np.ascontiguousarray(np.asarray(embeddings), dtype=np.float32)
    lab = np.asarray(labels).astype(np.int64)
    B, D = X.shape
    assert B == B_TOTAL and D == D_MODEL, (B, D)

    sched = _schedule(lab)
    if sched is None:
        return _host_fallback(X, lab)
    blocks, T, NT = sched
    in_maps = _make_in_maps(X, lab, blocks, T, NT)

    key = (B, D, T, NT)
    nc = _PROGRAM_CACHE.get(key)
    if nc is None:
        nc = _build_program(B, D, T, NT)
        _PROGRAM_CACHE[key] = nc

    res = run_bass_kernel_spmd(nc, in_maps, core_ids=list(range(N_CORES)))
    total_sum = 0.0
    total_cnt = 0.0
    for r in res.results:
        o = np.asarray(r["out"], dtype=np.float64)
        total_sum += o[0, 0]
        total_cnt += o[0, 1]
    return np.float32(total_sum / (total_cnt + EPS))


# revision 16
# speedup vs baseline: 1.2973x; 1.2881x over previous
"""Batch-all triplet loss on 8 Trainium2 NeuronCores (Bass/Tile).

Math: with d = pairwise euclidean distance matrix of the B embeddings,
  loss = sum_{i,j,k valid} relu(d[i,j] - d[i,k] + margin) / (#positive + eps)
valid <=> i != j, labels[i] == labels[j], labels[i] != labels[k]
(the other distinctness constraints are implied by the label ones).

Sharding: anchors are grouped by class; each core hosts 2 classes in two
64-row blocks (data-driven gathers keep the single SPMD program uniform).

Per core, on device:
  prep:
  - one fused matmul group produces g[r, c] = dot(x_r, x_c) - 0.5*sq_c
    - 0.5*sq_r - MASK*[class(c)==class(r)] : the X contraction runs in
    fp8 (4x128 K-chunks; the stochastic fp8 dot error ~0.02 on d is far
    inside the 2e-2 gate and halves the gating DMA bytes); a 20-row bf16
    tail chunk carries the anchor squared norm (hi+lo, stationary data
    vs moving ones), the column squared norm (hi+lo, moving data vs
    stationary ones) and 16 class-mask rows (+-240 products).
  - ACT Relu(scale=-2) then ACT Sqrt gives dmat (masked columns ~338);
    DVE tensor_scalar(-SHIFT) emits rhs_c[:, 0:B] (bf16, recentred).
  - the same structure over the partner columns gives bias[r, t] =
    d(anchor_r, t-th member of r's class) + margin - SHIFT at
    rhs_c[:, B:B+T] (bf16).
  pair loop, NT tiles of 128 (anchor,positive) pairs (software-pipelined
  so each engine's in-order stream never stalls):
  - PE replicates each pair's anchor row + bias row with a one-hot
    matmul into PSUM: rep[p, 0:B+T] = rhs_c[anchor(p), :].
  - DVE builds the bias-select mask on the fly (sel[p,t] =
    (t == pos_idx[p, tau]), 4x mode) -> scalar_tensor_tensor extracts
    bias_p; ACT Relu(scale=-1, bias=bias_p) -> o1 (bf16 SBUF);
    DVE is_gt -> o2 (bf16, 4x mode).
  - PE reduces o1 and o2 with ones-matmuls into two persistent PSUM
    accumulator regions (column-wrapped, one accumulation group each
    across all NT tiles) - the DVE reduce path is 1x-capped and slower.
  - invalid negatives contribute 0 (the +306 mask), padded pairs are
    all-zero rows with bias 0 and contribute 0 to both sums.
  tail: one strided tensor_reduce of the [1, 2x512] accumulator ->
  [1, 2] (sum, count) DMA'd out; host adds the 8 cores up.
"""

import numpy as np

import bass_rust
import concourse.bass as bass
import concourse.tile as tile
from concourse import mybir
from concourse.bass_utils import run_bass_kernel_spmd

N_CORES = 8
D_MODEL = 512
B_TOTAL = 640
N_CLASSES = 16
MARGIN = 0.3
EPS = 1e-8
RB = 64  # rows per block (max class size the device path supports)
MASKQ = 240.0  # class-mask factor; product 57600 pushes masked d to ~338
SHIFT = 32.0  # d values live in ~[27.6, 37]; recentring helps bf16
AUGR = 4 + N_CLASSES  # sqa_hi, sqa_lo, norm_hi, norm_lo, class masks
TP = 48  # padded iota/sel width (even for DVE 4x mode)
F32 = mybir.dt.float32
BF16 = mybir.dt.bfloat16
FP8 = mybir.dt.float8e4
NP_BF16 = mybir.dt.np(mybir.dt.bfloat16)
NP_FP8 = mybir.dt.np(mybir.dt.float8e4)

_PROGRAM_CACHE: dict = {}


def _split_multi_waits(nc):
    """This toolchain's walrus codegen supports only ONE sync-wait per
    instruction; Tile can emit several. Move the extra waits onto
    same-engine NoOps inserted immediately before the instruction."""
    for func in nc.m.functions:
        for block in func.blocks:
            out = []
            for inst in block.instructions:
                si = inst.sync_info
                waits = list(si.on_wait) if si else []
                if len(waits) > 1:
                    for j, w in enumerate(waits[:-1]):
                        nop = mybir.InstNoOp(
                            name=f"{inst.name}-wsplit{j}", ins=[], outs=[]
                        )
                        nop.engine = inst.engine
                        nop.sync_info = bass_rust.SyncInfo(on_wait=[w], on_update=[])
                        out.append(nop)
                    inst.sync_info = bass_rust.SyncInfo(
                        on_wait=[waits[-1]], on_update=list(si.on_update)
                    )
                out.append(inst)
            block.instructions = out


def _build_program(B: int, D: int, T: int, NT: int):
    """One SPMD program for all 8 cores; per-core behavior is data-driven."""
    nc = bass.Bass()

    # xcombo columns: [0:B) moving operand (X^T), [B:B+128) stationary
    # anchor gather, [B+128:B+128+2T) positive-partner gather, all fp8.
    # Packed DRAM layout [128, 4*W]: chunk c of the K=512 contraction
    # occupies cols [c*W:(c+1)*W) (K-row c*128+p lives in partition p).
    W = B + 128 + 2 * T
    xcombo = nc.declare_dram_parameter("xcombo", [128, 4 * W], FP8, isOutput=False)
    aug = nc.declare_dram_parameter("aug", [AUGR, W], BF16, isOutput=False)
    # one-hot stationary, shipped with only the 2*T meaningful rows
    onehot = nc.declare_dram_parameter(
        "onehot", [2 * T, NT * 128], BF16, isOutput=False
    )
    selm = nc.declare_dram_parameter("selm", [128, NT * TP], BF16, isOutput=False)
    out_d = nc.declare_dram_parameter("out", [1, 2], F32, isOutput=True)

    WR = B + T  # replicated tile: negative columns + bias columns
    L0, L1 = B, B + 128  # stationary (anchor) columns within a chunk
    P0 = B + 128  # partner columns within a chunk

    with tile.TileContext(nc) as tc:
        with (
            tc.tile_pool(name="const", bufs=1) as const,
            tc.tile_pool(name="work", bufs=1) as work,
        ):
            # preload the ACT table set while DMAs run
            warm = const.tile([1, 8], F32)
            nc.vector.memset(warm, 1.0)
            nc.scalar.activation(out=warm, in_=warm,
                                 func=mybir.ActivationFunctionType.Sqrt)
            nc.scalar.activation(out=warm, in_=warm,
                                 func=mybir.ActivationFunctionType.Relu)

            # ---- input DMAs. sync's queue carries the big streams in
            # consumption order (xcombo gates prep; one-hot and sel
            # chunks are consumed progressively by the pair loop);
            # scalar's queue carries the small aug table in parallel. ----
            ktile = const.tile([128, 4 * W], FP8)
            nc.sync.dma_start(out=ktile, in_=xcombo[:, :])
            taug = const.tile([AUGR, W], BF16)
            nc.scalar.dma_start(out=taug, in_=aug[:, :])
            t_oh = const.tile([128, NT * 128], BF16)
            if T < RB:  # zero the pad rows the gathers skip
                nc.vector.memset(t_oh, 0.0)
            t_sel = const.tile([128, NT * TP], BF16)
            ohq = (NT + 3) // 4 * 128  # one-hot cols per quarter-chunk
            slq = (NT + 1) // 2 * TP  # sel cols per half-chunk
            for blk in range(2):  # [oh-q0, sel-h0, oh-q1], [oh-q2, sel-h1, oh-q3]
                c0, c1 = 2 * blk * ohq, min((2 * blk + 1) * ohq, NT * 128)
                nc.sync.dma_start(out=t_oh[0:T, c0:c1], in_=onehot[0:T, c0:c1])
                nc.sync.dma_start(
                    out=t_oh[RB : RB + T, c0:c1], in_=onehot[T : 2 * T, c0:c1]
                )
                s0, s1 = blk * slq, min((blk + 1) * slq, NT * TP)
                nc.sync.dma_start(out=t_sel[:, s0:s1], in_=selm[:, s0:s1])
                c0, c1 = (2 * blk + 1) * ohq, min((2 * blk + 2) * ohq, NT * 128)
                if c0 < c1:
                    nc.sync.dma_start(out=t_oh[0:T, c0:c1], in_=onehot[0:T, c0:c1])
                    nc.sync.dma_start(
                        out=t_oh[RB : RB + T, c0:c1], in_=onehot[T : 2 * T, c0:c1]
                    )

            kc = ktile.rearrange("p (c w) -> p c w", c=4)

            rhs_c = work.tile([128, WR], BF16)  # [d_in | bias] combined

            with tc.tile_pool(name="psum_prep", bufs=1, space="PSUM") as psp:
                # ---- g = dot - 0.5*sq_col - 0.5*sq_anchor - class masks ----
                g = psp.tile([128, 1024], F32)
                for n0, n1 in [(0, 512), (512, B)]:
                    for ki in range(4):
                        nc.tensor.matmul(
                            g[:, n0:n1],
                            kc[:, ki, L0:L1],
                            kc[:, ki, n0:n1],
                            start=(ki == 0),
                            stop=False,
                        )
                    nc.tensor.matmul(
                        g[:, n0:n1], taug[:, L0:L1], taug[:, n0:n1],
                        start=False, stop=True,
                    )
                # ---- positive-pair distances: pb[r, t] ----
                pb = psp.tile([128, T], F32)
                for blk in range(2):
                    r0, r1 = blk * RB, (blk + 1) * RB
                    c0, c1 = P0 + blk * T, P0 + (blk + 1) * T
                    for ki in range(4):
                        nc.tensor.matmul(
                            pb[r0:r1, :],
                            kc[:, ki, L0 + r0 : L0 + r1],
                            kc[:, ki, c0:c1],
                            start=(ki == 0),
                            stop=False,
                        )
                    nc.tensor.matmul(
                        pb[r0:r1, :], taug[:, L0 + r0 : L0 + r1],
                        taug[:, c0:c1], start=False, stop=True,
                    )
                # bias = d_pos + margin - SHIFT -> rhs_c[:, B:B+T] (bf16)
                bsq = work.tile([128, T], F32)
                nc.scalar.activation(
                    out=bsq, in_=pb, func=mybir.ActivationFunctionType.Relu,
                    scale=-2.0,
                )
                bd = work.tile([128, T], F32)
                nc.scalar.activation(
                    out=bd, in_=bsq, func=mybir.ActivationFunctionType.Sqrt,
                )
                nc.vector.tensor_scalar(
                    out=rhs_c[:, B : B + T], in0=bd,
                    scalar1=float(MARGIN - SHIFT), scalar2=None,
                    op0=mybir.AluOpType.add,
                )
                # d_in = d - SHIFT (masked columns stay ~306)
                dsq = work.tile([128, B], F32)
                nc.scalar.activation(
                    out=dsq, in_=g[:, 0:B],
                    func=mybir.ActivationFunctionType.Relu, scale=-2.0,
                )
                dmat = work.tile([128, B], F32)
                nc.scalar.activation(
                    out=dmat, in_=dsq, func=mybir.ActivationFunctionType.Sqrt,
                )
                nc.vector.tensor_scalar(
                    out=rhs_c[:, 0:B], in0=dmat,
                    scalar1=-SHIFT, scalar2=None,
                    op0=mybir.AluOpType.add,
                )

            # ---- pair loop (software-pipelined: the count-accumulate of
            # tile t is emitted during tile t+1 so the in-order DVE
            # stream never waits on ACT) ----
            sum_ps = work.tile([128, NT], F32)
            cacc = work.tile([128, B], BF16)  # spatial count accumulator
            nc.vector.memset(cacc, 0.0)
            with (
                tc.tile_pool(name="psum_loop", bufs=3, space="PSUM") as psl,
                tc.tile_pool(name="psum_tail", bufs=1, space="PSUM") as pst,
                tc.tile_pool(name="bp", bufs=3) as bpp,
                tc.tile_pool(name="ttrp", bufs=2) as ttrp,
                tc.tile_pool(name="o1p", bufs=3) as o1p,
            ):
                o1s = {}

                def emit_cnt(t):
                    # cacc += (o1 > 0), fused in one in-place 2x STT
                    nc.vector.scalar_tensor_tensor(
                        out=cacc, in0=o1s.pop(t), scalar=0.0,
                        in1=cacc,
                        op0=mybir.AluOpType.is_gt, op1=mybir.AluOpType.add,
                    )

                for tau in range(NT):
                    # PE: replicate anchor row + bias row per pair
                    rep = psl.tile([128, 1024], F32, tag="rep")
                    oh = t_oh[:, tau * 128 : (tau + 1) * 128]
                    nc.tensor.matmul(
                        rep[:, 0:512], oh, rhs_c[:, 0:512],
                        start=True, stop=True,
                    )
                    nc.tensor.matmul(
                        rep[:, 512:WR], oh, rhs_c[:, 512:WR],
                        start=True, stop=True,
                    )
                    # DVE: extract this pair's bias
                    bias_p = bpp.tile([128, 1], F32, tag="bias_p")
                    ttr = ttrp.tile([128, T], F32, tag="ttr")
                    nc.vector.scalar_tensor_tensor(
                        out=ttr, in0=rep[:, B:WR], scalar=1.0,
                        in1=t_sel[:, tau * TP : tau * TP + T],
                        op0=mybir.AluOpType.mult, op1=mybir.AluOpType.mult,
                        accum_out=bias_p,
                    )
                    # ACT: o1 = relu(bias_p - rep) (bf16) + sum accumulator
                    o1 = o1p.tile([128, B], BF16, tag="o1")
                    nc.scalar.activation(
                        out=o1, in_=rep[:, 0:B],
                        func=mybir.ActivationFunctionType.Relu,
                        bias=bias_p, scale=-1.0,
                        accum_out=sum_ps[:, tau : tau + 1],
                    )
                    o1s[tau] = o1
                    if tau >= 1:
                        emit_cnt(tau - 1)
                emit_cnt(NT - 1)

                # ---- tail: reduce -> [128,2] -> ones-matmul -> [1,2] ----
                stat = work.tile([128, 2], F32)
                nc.vector.tensor_reduce(
                    out=stat[:, 0:1], in_=sum_ps, axis=mybir.AxisListType.X,
                    op=mybir.AluOpType.add,
                )
                scr = work.tile([128, B], BF16)
                nc.vector.tensor_scalar(
                    out=scr, in0=cacc, scalar1=1.0, scalar2=0.0,
                    op0=mybir.AluOpType.mult, op1=mybir.AluOpType.add,
                    accum_out=stat[:, 1:2],
                )
                onesf = work.tile([128, 1], F32)
                nc.vector.memset(onesf, 1.0)
                tot = pst.tile([1, 2], F32)
                nc.tensor.matmul(tot, onesf, stat, start=True, stop=True)
                tot_s = work.tile([1, 2], F32)
                nc.vector.tensor_copy(tot_s, tot)
                nc.sync.dma_start(out=out_d[:, :], in_=tot_s)

    _split_multi_waits(nc)
    return nc


def _schedule(labels: np.ndarray):
    """Group anchors by class, pair classes onto cores (big with small)."""
    vals, counts = np.unique(labels, return_counts=True)
    classes = [np.nonzero(labels == v)[0] for v in vals]
    order = np.argsort(-counts, kind="stable")
    classes = [classes[i] for i in order]
    sizes = [len(c) for c in classes]
    if len(classes) > 2 * N_CORES or max(sizes) > RB:
        return None  # device path infeasible for this label layout
    while len(classes) < 2 * N_CORES:
        classes.append(np.zeros((0,), dtype=np.int64))
    blocks = []
    for i in range(N_CORES):
        blocks.append((classes[i], classes[2 * N_CORES - 1 - i]))
    T = max(1, max(len(c) for c, _ in blocks))
    npairs = [len(a) * (len(a) - 1) + len(b) * (len(b) - 1) for a, b in blocks]
    NT = max(1, (max(npairs) + 127) // 128)
    return blocks, T, NT


def _host_fallback(X: np.ndarray, labels: np.ndarray) -> np.float32:
    """Exact numpy implementation (only for label layouts the device
    schedule cannot represent — cannot occur for randint(0,16) labels)."""
    Xd = X.astype(np.float64)
    dot = Xd @ Xd.T
    sq = np.diag(dot).copy()
    dm = np.maximum(sq[None, :] - 2.0 * dot + sq[:, None], 0.0)
    zero = dm == 0.0
    dm = np.sqrt(dm + zero * EPS) * (1.0 - zero)
    total = 0.0
    npos = 0
    B = len(labels)
    for i in range(B):
        pos = (labels == labels[i]) & (np.arange(B) != i)
        neg = labels != labels[i]
        p = dm[i, pos] + MARGIN
        n = dm[i, neg]
        tl = np.maximum(p[:, None] - n[None, :], 0.0)
        total += tl.sum()
        npos += (tl > EPS).sum()
    return np.float32(total / (npos + EPS))


def _make_in_maps(X: np.ndarray, lab: np.ndarray, blocks, T: int, NT: int):
    B, D = X.shape
    sq = (X.astype(np.float64) ** 2).sum(axis=1).astype(np.float32)
    W = B + 128 + 2 * T

    X8 = X.astype(NP_FP8)
    XT8 = np.ascontiguousarray(X8.T)
    sq_hi = (-0.5 * sq).astype(NP_BF16)
    sq_lo = ((-0.5 * sq) - sq_hi.astype(np.float32)).astype(NP_BF16)
    in_maps = []
    for core in range(N_CORES):
        cls_a, cls_b = blocks[core]
        row_idx = np.zeros(128, dtype=np.int64)
        for blk, cls in enumerate((cls_a, cls_b)):
            m = len(cls)
            r0 = blk * RB
            if m:
                row_idx[r0 : r0 + m] = cls
                row_idx[r0 + m : r0 + RB] = cls[0]

        par_idx = np.zeros(2 * T, dtype=np.int64)
        for blk, cls in enumerate((cls_a, cls_b)):
            m = len(cls)
            if m:
                par_idx[blk * T : blk * T + m] = cls

        # X part of the contraction (fp8), packed [128, 4, W]
        xcols = np.concatenate([XT8, XT8[:, row_idx], XT8[:, par_idx]], axis=1)
        packed = np.ascontiguousarray(
            xcols.reshape(4, 128, W).transpose(1, 0, 2)
        ).reshape(128, 4 * W)

        # aug rows (bf16): anchor norms (stationary data x moving ones),
        # column norms (stationary ones x moving data), 16 class-mask
        # rows (stationary 240*onehot x moving -240 on own-class columns)
        L0, L1 = B, B + 128
        augt = np.zeros((AUGR, W), dtype=np.float32)
        augt[0:4, :] = 1.0
        augt[0, L0:L1] = sq_hi[row_idx].astype(np.float32)
        augt[1, L0:L1] = (
            -0.5 * sq[row_idx] - sq_hi[row_idx].astype(np.float32)
        )
        augt[2, 0:B] = sq_hi.astype(np.float32)
        augt[3, 0:B] = sq_lo.astype(np.float32)
        augt[2, L1:W] = sq_hi[par_idx].astype(np.float32)
        augt[3, L1:W] = sq_lo[par_idx].astype(np.float32)
        for c in range(N_CLASSES):
            augt[4 + c, 0:B] = np.where(lab == c, -MASKQ, 0.0)
            augt[4 + c, L0:L1] = (lab[row_idx] == c) * MASKQ
        aug_bf = augt.astype(NP_BF16)
        # keep the anchor-norm hi+lo split exact after the bf16 round
        aug_bf[1, L0:L1] = (
            -0.5 * sq[row_idx] - aug_bf[0, L0:L1].astype(np.float32)
        ).astype(NP_BF16)

        # pair tables: one-hot anchor pick (compact: 2*T used rows) and
        # the bias-column select mask
        onehot = np.zeros((2 * T, NT * 128), dtype=NP_BF16)
        selm = np.zeros((128, NT * TP), dtype=NP_BF16)
        p = 0
        for blk, cls in enumerate((cls_a, cls_b)):
            m = len(cls)
            for i in range(m):
                for t in range(m):
                    if t == i:
                        continue
                    tau, q = divmod(p, 128)
                    onehot[blk * T + i, tau * 128 + q] = 1.0
                    selm[q, tau * TP + t] = 1.0
                    p += 1
        assert p <= NT * 128

        in_maps.append(
            {"xcombo": packed, "aug": aug_bf, "selm": selm, "onehot": onehot}
        )
    return in_maps


def kernel(embeddings: np.ndarray, labels: np.ndarray) -> np.ndarray:
    X = np.ascontiguousarray(np.asarray(embeddings), dtype=np.float32)
    lab = np.asarray(labels).astype(np.int64)
    B, D = X.shape
    assert B == B_TOTAL and D == D_MODEL, (B, D)

    sched = _schedule(lab)
    if sched is None:
        return _host_fallback(X, lab)
    blocks, T, NT = sched
    in_maps = _make_in_maps(X, lab, blocks, T, NT)

    key = (B, D, T, NT)
    nc = _PROGRAM_CACHE.get(key)
    if nc is None:
        nc = _build_program(B, D, T, NT)
        _PROGRAM_CACHE[key] = nc

    res = run_bass_kernel_spmd(nc, in_maps, core_ids=list(range(N_CORES)))
    total_sum = 0.0
    total_cnt = 0.0
    for r in res.results:
        o = np.asarray(r["out"], dtype=np.float64)
        total_sum += o[0, 0]
        total_cnt += o[0, 1]
    return np.float32(total_sum / (total_cnt + EPS))
